# revision 46
# baseline (speedup 1.0000x reference)
"""DispersionLoss kernel for Trainium2 (8 NeuronCores, Bass/Tile).

Reference computation (N=16384, F=64, K=32, C=128):
    bin_mass[f,k]  = sum_n m[n,f,k] + EPS
    SWY[f,k,c]     = sum_n m[n,f,k] * y[n,c]
    cent[f,k,c]    = SWY / bin_mass
    loss_dispersion= sum_fk (A/bin_mass - c_sq)   [EPS*c_sq/bin_mass ~1e-11, dropped]
        where A[f,k] = sum_n m[n,f,k]*|y_n|^2
    loss_entropy   = sum_fk p*log(p+EPS), p = bin_mass/N
    loss_repulsion = sum_f sum_k exp(-|cent[f,k]-cent[f,k+1]|^2)
    loss_inter     = sum_f (sum_{kj} exp(-pairwise) - K) / 2 / F

Sharding: over F (8 features per core) -> every loss term decomposes per-f,
no cross-core collectives; host sums 8 partial vectors.

Design:
  - inputs quantized to fp8 e4m3 on host; ysq precomputed on host (f32) and
    shipped as an fp8 hi+lo pair -> device does zero prep work.  ~6.1 MiB/core.
  - all input DMA on the sync queue in consumption order (y chunks
    interleaved between g blocks; first chunks smaller to start compute
    sooner).  gpsimd's software-DGE path is avoided.
  - ~110 dummy id16 matmuls run while DMA streams so the PE HAM clock gate
    is at 2.4 GHz when the real matmuls start.
  - G-stationary DoubleRow fp8 matmuls: for each 256-row pair u and bin-half
    h, ps[h] += g[u,:,h-half].T @ [Y | 1 | ysq_h | ysq_l].  G enters the PE
    once; mass/A ride along as extra moving columns; output is bin-major.
  - tail: scalar engine touches only Copy/Identity/Ln/Exp (Exp+Ln tables
    preloaded).  Pairwise matrix gets a -B block bias on cross-feature
    entries (K=8 indicator matmul) so exp() zeroes them and the Exp ACT's
    accum_out yields the inter-loss block sums for free.  Repulsion comes
    from adjacent-column differences of the transposed centered centroids.
"""

import numpy as np

N = 16384
F = 64
K = 32
C = 128
NCORES = 8
F_PER_CORE = F // NCORES          # 8
FK = F_PER_CORE * K               # 256 bins per core
NPAIR = N // 256                  # 64 subtile pairs (DoubleRow: 256 rows/mm)
W = 132                           # moving cols: [y(128) | 1 | ysq_h | ysq_l | pad]
GB = 8                            # pairs per g DMA block
NGB = NPAIR // GB                 # 8 g blocks
CSC = 16.0                        # centered-centroid scale (keeps fp16 normal)
BBIAS = 3840.0                    # cross-feature psE bias: exp arg -= 30
NWARM = 56                        # PE warm-up matmuls; also delays phase-1
                                  # start so DMA builds a backlog and the
                                  # PE streams at 100% duty (no re-throttle)

LAMBDA_ENTROPY = 0.1
LAMBDA_REPULSION = 0.5
LAMBDA_INTER = 0.3
EPS = 1e-8

_NC_CACHE = {}


def _f8dtype():
    import ml_dtypes
    return ml_dtypes.float8_e4m3


def _pack_g(gc: np.ndarray) -> np.ndarray:
    """(N, FK) fp8 -> (NGB*128, GB*2*FK): block b row p holds, for the 8
    pairs u of the block, [i=0 | i=1] x FK cols where the n-row is
    256*u + 128*i + p."""
    x = gc.reshape(NPAIR, 2, 128, FK).transpose(2, 0, 1, 3)   # p, u, i, fk
    x = x.reshape(128, NPAIR * 2 * FK).reshape(128, NGB, GB * 2 * FK)
    return np.ascontiguousarray(x.transpose(1, 0, 2).reshape(NGB * 128, GB * 2 * FK))


def _pack_y(yslab: np.ndarray) -> np.ndarray:
    """(N, W) fp8 -> (128, NPAIR*2*W): partition p holds pair-major slabs."""
    return np.ascontiguousarray(
        yslab.reshape(NPAIR, 2, 128, W).transpose(2, 0, 1, 3).reshape(128, NPAIR * 2 * W)
    )


def _finalize(parts: np.ndarray):
    """parts: (ncores, 8) = [wv0, wv1, mlg0, mlg1, eall0, eall1, reptot, repx]."""
    r = parts.astype(np.float64).sum(axis=0)
    disp = r[0] + r[1]
    ent = (r[2] + r[3]) / N
    rep = r[6] - r[7]
    inter = (r[4] + r[5] - F * K) / (2.0 * F)
    tot = disp + LAMBDA_ENTROPY * ent + LAMBDA_REPULSION * rep + LAMBDA_INTER * inter
    return tuple(np.float32(v) for v in (tot, disp, ent, rep, inter))


def _build_nc():
    import concourse.bacc as bacc
    import concourse.tile as tile
    from concourse import mybir

    f32 = mybir.dt.float32
    f16 = mybir.dt.float16
    f8 = mybir.dt.float8e4
    DR = mybir.MatmulPerfMode.DoubleRow
    AF = mybir.ActivationFunctionType
    OP = mybir.AluOpType

    nc = bacc.Bacc("TRN2", target_bir_lowering=False, debug=False,
                   enable_asserts=False, enable_partition_id=False)
    g_dram = nc.dram_tensor("g", (NGB * 128, GB * 2 * FK), f8, kind="ExternalInput").ap()
    y_dram = nc.dram_tensor("y", (128, NPAIR * 2 * W), f8, kind="ExternalInput").ap()
    out_dram = nc.dram_tensor("out", (1, 8), f32, kind="ExternalOutput").ap()

    with tile.TileContext(nc) as tc:
        with (
            tc.tile_pool(name="singles", bufs=1) as singles,
            tc.tile_pool(name="gpool", bufs=8) as gpool,
            tc.tile_pool(name="scr", bufs=2) as scr,
            tc.tile_pool(name="ph2", bufs=1) as ph2,
            tc.tile_pool(name="psacc", bufs=1, space="PSUM") as psacc,
            tc.tile_pool(name="pstmp", bufs=1, space="PSUM") as pstmp,
        ):
            yres = singles.tile([128, NPAIR * 2 * W], f8, name="yres")

            def emit_ychunk(plo, phi):
                lo = plo * 2 * W
                hi = phi * 2 * W
                nc.sync.dma_start(out=yres[:, lo:hi], in_=y_dram[:, lo:hi])

            # ---- constants ----
            ones128 = singles.tile([128, 1], f32)
            nc.gpsimd.memset(ones128, 1.0)
            eps128 = singles.tile([128, 1], f32)
            nc.gpsimd.memset(eps128, EPS)
            ones16c = singles.tile([128, 1], f16)
            nc.gpsimd.memset(ones16c, 1.0)
            id16 = singles.tile([128, 128], f16)
            nc.gpsimd.memset(id16, 0.0)
            nc.gpsimd.affine_select(
                out=id16, in_=id16,
                compare_op=OP.not_equal,
                fill=1.0, base=0, pattern=[[-1, 128]], channel_multiplier=1,
            )
            ones_row = singles.tile([1, FK], f16)
            nc.gpsimd.memset(ones_row, 1.0)
            mhalf16 = singles.tile([128, 1], f16)
            nc.gpsimd.memset(mhalf16, -0.5)
            qneg_sb = singles.tile([1, FK], f16)
            # feature-indicator [8, FK] and cross-feature bias -B*(1-ind)
            ind16 = singles.tile([8, FK], f16)
            nc.gpsimd.memset(ind16, 0.0)
            i3 = ind16.rearrange("p (blk c) -> p blk c", c=32)
            nc.gpsimd.affine_select(
                out=i3, in_=i3, compare_op=OP.not_equal,
                fill=1.0, base=0, pattern=[[1, 8], [0, 32]],
                channel_multiplier=-1,
            )
            indB = singles.tile([8, FK], f16)
            nc.gpsimd.memset(indB, -BBIAS)
            b3 = indB.rearrange("p (blk c) -> p blk c", c=32)
            nc.gpsimd.affine_select(
                out=b3, in_=b3, compare_op=OP.not_equal,
                fill=0.0, base=0, pattern=[[1, 8], [0, 32]],
                channel_multiplier=-1,
            )
            # st cols: [wv0, wv1, m*ln0, m*ln1, eall0, eall1, reptot, repx]
            st = ph2.tile([128, 8], f32)
            nc.gpsimd.memset(st, 0.0)

            # ---- PE clock warm-up: keep the HAM gate open until real MMs ----
            wps = pstmp.tile([128, 128], f32, tag="psT0", name="warmps")
            for _ in range(NWARM):
                nc.tensor.matmul(wps, id16, id16, start=True, stop=True)

            # ---- preload Exp+Ln tables.  The compiler trusts only the most
            # recent load: warm Ln LAST so the tail LNs are free; the single
            # unavoidable Exp reload is absorbed by an in-tail decoy.
            warm = ph2.tile([1, 2], f32)
            nc.scalar.activation(out=warm[0:1, 0:1], in_=ones128[0:1, 0:1], func=AF.Exp)
            nc.scalar.activation(out=warm[0:1, 1:2], in_=ones128[0:1, 0:1], func=AF.Ln)

            def emit_dummies(n):
                dw = pstmp.tile([128, FK], f32, tag="psE0", name="dw")
                for _ in range(n):
                    nc.tensor.matmul(dw, ones_row[0:1, 0:128], ones_row,
                                     start=True, stop=True)

            # ---- phase 1: G-stationary DoubleRow accumulation ----
            # ps[h][:, 0:128]=SWY, [:,128]=mass_raw, [:,129:131]=A_hi/lo
            ps = [psacc.tile([128, W], f32, name=f"acc{h}") for h in range(2)]
            # sync-queue emission order == consumption order
            YS = {0: (0, 8), 1: (8, 16), 2: (16, 32), 4: (32, 48), 6: (48, 64)}

            def emit_mm(u, gv, yv, h):
                nc.tensor.matmul(
                    ps[h], gv[:, :, h * 128:(h + 1) * 128], yv,
                    start=(u == 0), stop=(u == NPAIR - 1), perf_mode=DR,
                )

            for b in range(NGB):
                if b in YS:
                    emit_ychunk(*YS[b])
                g = gpool.tile([128, GB * 2 * FK], f8)
                nc.sync.dma_start(out=g, in_=g_dram[b * 128:(b + 1) * 128, :])
                views = []
                for ul in range(GB):
                    u = b * GB + ul
                    gv = g[:, ul * 2 * FK:(ul + 1) * 2 * FK].rearrange(
                        "p (i fk) -> p i fk", i=2)
                    yv = yres[:, u * 2 * W:(u + 1) * 2 * W].rearrange(
                        "p (i w) -> p i w", i=2)
                    views.append((u, gv, yv))
                if b < NGB - 1:
                    for u, gv, yv in views:
                        emit_mm(u, gv, yv, 0)
                        emit_mm(u, gv, yv, 1)
                else:
                    # finish group h=0 first so its stats start earlier
                    for u, gv, yv in views:
                        emit_mm(u, gv, yv, 0)
                    for u, gv, yv in views:
                        emit_mm(u, gv, yv, 1)

            # ---- tail ----
            mass = ph2.tile([128, 2], f32)
            inv = ph2.tile([128, 2], f32)
            a_ = ph2.tile([128, 2], f32)
            csq = ph2.tile([128, 2], f32)
            t1 = ph2.tile([128, 2], f32)
            lg = ph2.tile([128, 2], f32)
            cent16 = ph2.tile([128, 2 * 128], f16)
            ccT = ph2.tile([128, FK], f16)
            nshift = ph2.tile([128, 1], f32)
            sqc = scr.tile([128, FK], f16, tag="sqc")
            sqf = scr.tile([128, FK], f16, tag="sqf")

            # critical path: mass -> inv -> cent16 -> transpose -> ccT -> q
            emit_dummies(9)          # hold the PE clock gate open during stats
            for h in range(2):
                nc.vector.tensor_scalar_add(
                    mass[:, h:h + 1], in0=ps[h][:, 128:129], scalar1=eps128)
            nc.vector.reciprocal(inv, mass)
            psT = []
            for h in range(2):
                with nc.allow_low_precision(reason="cent fp16 for exp terms"):
                    nc.scalar.activation(
                        out=cent16[:, h * 128:(h + 1) * 128],
                        in_=ps[h][:, 0:128], func=AF.Copy,
                        scale=inv[:, h:h + 1])
                ps_t = pstmp.tile([128, 128], f32, tag=f"psT{h}", name=f"psT{h}")
                nc.tensor.matmul(ps_t, cent16[:, h * 128:(h + 1) * 128], id16,
                                 start=True, stop=True)
                psT.append(ps_t)
            emit_dummies(8)
            nc.vector.tensor_scalar_mul(nshift, in0=psT[0][:, 0:1], scalar1=-CSC)
            for h in range(2):
                with nc.allow_low_precision(reason="cc fp16 for exp terms"):
                    nc.scalar.activation(
                        out=ccT[:, h * 128:(h + 1) * 128], in_=psT[h],
                        func=AF.Identity, bias=nshift, scale=CSC)
            with nc.allow_low_precision(reason="scaled cc^2 fits fp16"):
                nc.vector.tensor_mul(sqf, ccT, ccT)
            # repulsion operand: dd = adjacent-column diffs of ccT (vector)
            dd = scr.tile([128, FK - 1], f16, tag="dd")
            with nc.allow_low_precision(reason="scaled cc diffs fp16"):
                nc.vector.tensor_sub(dd, ccT[:, 0:FK - 1], ccT[:, 1:FK])
                nc.vector.tensor_mul(dd, dd, dd)
            # ps_q = -q/2 directly (lhsT = -0.5 column)
            ps_q = pstmp.tile([1, FK], f32, tag="psq")
            nc.tensor.matmul(ps_q, mhalf16, sqf, start=True, stop=True)
            ps_nd_t = pstmp.tile([1, FK], f32, tag="psq", name="psnd")
            ps_nd = ps_nd_t[0:1, 0:FK - 1]
            # dots + cross-feature bias first (they need only ccT)
            ps_e = []
            for h in range(2):
                pe = pstmp.tile([128, FK], f32, tag=f"psE{h}", name=f"psE{h}")
                nc.tensor.matmul(pe, ccT[:, h * 128:(h + 1) * 128], ccT,
                                 start=True, stop=False)
                nc.tensor.matmul(pe, ind16[:, h * 128:(h + 1) * 128], indB,
                                 start=False, stop=False)
                ps_e.append(pe)
            # qneg first (rank-1 critical path), then Ln in the scalar gap,
            # then the decoy Exp that soaks up the one Exp-table reload
            # while the PE runs the rank-1/bias matmuls.
            with nc.allow_low_precision(reason="q fp16 rank-1 operand"):
                nc.scalar.activation(out=qneg_sb, in_=ps_q, func=AF.Copy)
            for h in range(2):
                nc.scalar.activation(out=lg[:, h:h + 1], in_=mass[:, h:h + 1],
                                     func=AF.Ln, bias=eps128, scale=1.0 / N)
            nc.scalar.activation(out=warm[0:1, 0:1], in_=psT[0][0:1, 0:1],
                                 func=AF.Exp, scale=2.0 / (CSC * CSC))
            nc.tensor.matmul(ps_nd, ones16c, dd, start=True, stop=True)
            for h in range(2):
                nc.tensor.matmul(ps_e[h], ones_row[0:1, 0:128], qneg_sb,
                                 start=False, stop=False)
                nc.tensor.matmul(ps_e[h], qneg_sb[0:1, h * 128:(h + 1) * 128],
                                 ones_row, start=False, stop=True)

            # exps: repulsion first (its input is ready earliest), then the
            # two pairwise exps whose accum_out gives the inter block sums
            en_row = ph2.tile([1, FK - 1], f32)
            nc.scalar.activation(out=en_row, in_=ps_nd, func=AF.Exp,
                                 scale=-1.0 / (CSC * CSC),
                                 accum_out=st[0:1, 6:7])
            for h in range(2):
                e_full = scr.tile([128, FK], f16, tag="efull", name=f"ef{h}")
                with nc.allow_low_precision(reason="E<=1 fp16; accum f32"):
                    nc.scalar.activation(out=e_full, in_=ps_e[h], func=AF.Exp,
                                         scale=2.0 / (CSC * CSC),
                                         accum_out=st[:, 4 + h:5 + h])
            # subtract the 7 feature-crossing pairs (k = 31 mod 32)
            xview = en_row[0:1, 31:31 + 224].rearrange("p (m c) -> p m c", c=32)
            nc.vector.reduce_sum(st[0:1, 7:8], xview[:, :, 0:1],
                                 axis=mybir.AxisListType.XY)

            # off-critical stats (emitted late so they don't delay ccT)
            for h in range(2):
                nc.vector.reduce_sum(
                    a_[:, h:h + 1],
                    ps[h][:, 129:131].rearrange("p (one c) -> p one c", one=1),
                    axis=mybir.AxisListType.X)
            with nc.allow_low_precision(reason="csq via fp16 cent"):
                nc.vector.tensor_mul(sqc, cent16, cent16)
            nc.vector.reduce_sum(
                csq, sqc.rearrange("p (h c) -> p h c", c=128),
                axis=mybir.AxisListType.X)
            nc.vector.tensor_mul(t1, a_, inv)
            nc.vector.tensor_sub(st[:, 0:2], t1, csq)
            nc.vector.tensor_mul(st[:, 2:4], mass, lg)

            ps_res = pstmp.tile([1, 8], f32, tag="psres")
            nc.tensor.matmul(ps_res, ones128, st, start=True, stop=True)
            res = ph2.tile([1, 8], f32)
            nc.vector.tensor_copy(res, ps_res)
            nc.sync.dma_start(out=out_dram, in_=res)

    nc.compile()
    return nc


def get_nc():
    if "v4" not in _NC_CACHE:
        _NC_CACHE["v4"] = _build_nc()
    return _NC_CACHE["v4"]


def kernel(membership: np.ndarray, teacher_preds: np.ndarray, _trace: bool = False):
    from concourse.bass_utils import run_bass_kernel_spmd

    f8 = _f8dtype()
    m = np.asarray(membership, dtype=np.float32).reshape(N, F * K)
    y32 = np.asarray(teacher_preds, dtype=np.float32)
    ysq = np.einsum("nc,nc->n", y32, y32, dtype=np.float64).astype(np.float32)
    ysq_h = ysq.astype(f8)
    ysq_l = (ysq - ysq_h.astype(np.float32)).astype(f8)
    yslab = np.zeros((N, W), dtype=f8)
    yslab[:, 0:C] = y32.astype(f8)
    yslab[:, C] = np.float32(1.0)
    yslab[:, C + 1] = ysq_h
    yslab[:, C + 2] = ysq_l
    ypacked = _pack_y(yslab)

    m8 = m.astype(f8)
    nc = get_nc()
    in_maps = []
    for i in range(NCORES):
        in_maps.append({
            "g": _pack_g(m8[:, i * FK:(i + 1) * FK]),
            "y": ypacked,
        })
    res = run_bass_kernel_spmd(
        nc, in_maps, core_ids=list(range(NCORES)), trace=_trace,
    )
    parts = np.stack(
        [np.asarray(res.results[i]["out"][0], dtype=np.float64) for i in range(NCORES)]
    )
    out = _finalize(parts)
    if _trace:
        return out, res
    return out


if __name__ == "__main__":
    rng = np.random.default_rng(0)
    mem = rng.random((N, F, K), dtype=np.float32)
    tp = rng.random((N, C), dtype=np.float32)
    print(kernel(mem, tp))
